# revision 39
# baseline (speedup 1.0000x reference)
"""LocallyConnected2d (3x3, stride 1) Trainium2 Bass kernel, v3.

Shapes: x [64,32,64,64] f32, weight [1,64,32,62,62,9] f32 -> out [64,64,62,62] f32.

Strategy (orientation-B / "flipped" PE structure):
  - Shard output rows (OH=62, padded 64) across 8 cores: 8 rows/core,
    processed as 4 pairs (h0=2p even -> PE column group 0-1 / PSUM
    partitions 0-63, h1 odd -> col group 2-3 / partitions 64-127).
  - Per x column c: stationary = X_c [96=(ki,i), 64=b] (LDWEIGHTS once),
    streamed = the weight block that consumes x[:,c]: up to N=192 columns
    (w=c-2..c, i.e. kj=2..0, each x 64 couts). This amortizes the serialized
    LDWEIGHTS cost over ~3x more streamed columns than v2's orientation
    (w-stationary), cutting PE time ~2.3x. The two h's of a pair use
    different PE column groups so their matmuls can overlap.
  - PSUM accumulation uses per-element pending-zero semantics: one chunk
    (8 w positions = one 2KB PSUM bank) gets start=True on its first matmul
    only; overlapping c-windows then accumulate correctly (first writer of
    each element overwrites, later writers accumulate).
  - Weights shipped int8 (per-(o,h,w)-row symmetric quantization), cast to
    bf16 during the SWDGE (gpsimd) DMA: halves the dominant HBM read traffic.
    Dequantization happens ON HOST (output x scale), costing nothing on-chip.
    rel l2 ~7e-3 (gate 2e-2). MODE "bf16" ships bf16 weights instead (~2e-3).
  - x bf16, 3-row-stacked per output row; out fp16 [pair,(hp,b),w,o].
"""

import sys

if "/opt/trn_rl_repo" not in sys.path:
    sys.path.insert(0, "/opt/trn_rl_repo")

import numpy as np

B = 64
CIN = 32
H = W = 64
OH = OW = 62
COUT = 64
NCORES = 8
RH = 8
NPAIR = 4

MODE = "fp8"
TRACE = False
LAST = None
FP8_MAX = 15.5  # ml_dtypes.finfo(float8_e3m4).max
X_FP8 = True  # ship x as float8_e3m4 (scaled by X_SCALE) instead of bf16
X_SCALE = 2.0  # power of two; host dequant folds 1/X_SCALE into output

_PROGRAMS = {}


def _build_program(repeat=1, mode=None, variant="full"):
    mode = mode or MODE
    import concourse.bacc as bacc
    import concourse.mybir as mybir
    from concourse.tile import TileContext

    fp32 = mybir.dt.float32
    fp16 = mybir.dt.float16
    bf16 = mybir.dt.bfloat16
    nc = bacc.Bacc(
        "TRN2", target_bir_lowering=False, debug=False, num_devices=NCORES
    )

    i8 = mode == "i8"
    fp8 = mode == "fp8"
    if i8:
        wdram_dt, wtile_dt = mybir.dt.int8, bf16
    elif fp8:
        wdram_dt = wtile_dt = mybir.dt.float8e3
    else:
        wdram_dt = wtile_dt = bf16
    NQ = NPAIR // 2
    # [q][p=(ki,i)][pq][hp][c][j3][o]; w = c-2+j3 (kj=2-j3), zero-padded
    wt = nc.declare_dram_parameter(
        "wt", [NQ, 96, 2, 2, W, 3, COUT], wdram_dt, isOutput=False
    )
    # [q][p=(ki,i)][pq][hp][b][w]
    x_dt = mybir.dt.float8e3 if (X_FP8 and not i8) else bf16
    xs = nc.declare_dram_parameter("xs", [NQ, 96, 2, 2, B, W], x_dt, isOutput=False)
    # [q][(hp,b)][pq][w][o]
    out = nc.declare_dram_parameter(
        "out", [NQ, 128, 2, OW, COUT], fp16, isOutput=True
    )

    # (w0, nw) chunks: one PSUM bank (8 w x 64 o fp32 = 2KB) each
    CHUNKS = [(w0, min(8, OW - w0)) for w0 in range(0, OW, 8)]

    with TileContext(nc) as tc:
        with (
            tc.tile_pool(name="wp", bufs=2) as wpool,
            tc.tile_pool(name="xp", bufs=2) as xpool,
            tc.tile_pool(name="op", bufs=2) as opool,
            tc.tile_pool(name="pp", bufs=4, space="PSUM") as ppool,
        ):
            for q in [q_ for _ in range(repeat) for q_ in range(NQ)]:
                xt = xpool.tile([96, 2, 2, B, W], x_dt, tag="x")
                nc.scalar.dma_start(out=xt[:], in_=xs[q])
                wtile = wpool.tile([96, 2, 2, W, 3, COUT], wtile_dt, tag="w")
                nc.sync.dma_start(out=wtile[:], in_=wt[q])
                ot = opool.tile([128, 2, OW, COUT], fp16, tag="o")
                for pq in range(2):
                    for w0, nw in CHUNKS:
                        ps = ppool.tile([128, 8, COUT], fp32, tag="ps")
                        clist = list(range(w0, min(w0 + nw + 2, W)))
                        if variant == "minpe":
                            clist = clist[:1]
                        for ci, c in enumerate(clist):
                            ws = max(w0, c - 2)
                            we = min(c, w0 + nw - 1)
                            if ws > we:
                                continue
                            j3a = ws - c + 2
                            j3b = we - c + 2
                            first = ci == 0
                            last = c == clist[-1] or variant == "minpe"
                            for hp in range(2):
                                pb = 64 * hp
                                nc.tensor.matmul(
                                    ps[pb : pb + 64, ws - w0 : we - w0 + 1, :],
                                    lhsT=xt[:, pq, hp, :, c],
                                    rhs=wtile[:, pq, hp, c, j3a : j3b + 1, :],
                                    start=first,
                                    stop=last,
                                    tile_position=(0, pb),
                                )
                        nc.vector.tensor_copy(
                            ot[:, pq, w0 : w0 + nw, :], ps[:, 0:nw, :]
                        )
                nc.gpsimd.dma_start(out=out[q], in_=ot[:])
    nc.compile()
    return nc


_HOST_SCALE = [None]  # set by _prep_inputs in i8 mode; [o, h, w] f32


def _prep_inputs(x, weight, mode=None):
    mode = mode or MODE
    import ml_dtypes

    x = np.ascontiguousarray(x, dtype=np.float32)
    weight = np.ascontiguousarray(weight, dtype=np.float32)
    i8 = mode == "i8"

    # ---- weights ----
    w6 = weight[0].reshape(COUT, CIN, OH, OW, 3, 3)  # o,i,h,w,ki,kj
    if i8:
        am = np.abs(w6).max(axis=(1, 4, 5))  # [o, h, w]
        am = np.maximum(am, 1e-30)
        q = 127.0 / am
        wq6 = (
            np.rint(w6 * q[:, None, :, :, None, None]).clip(-127, 127).astype(np.int8)
        )
        _HOST_SCALE[0] = (am / 127.0).astype(np.float32)  # [o, h, w]
        src, dt = wq6, np.int8
    elif mode == "fp8":
        am = np.abs(w6).max(axis=(1, 4, 5))  # [o, h, w]
        am = np.maximum(am, 1e-30)
        sc = (am / FP8_MAX).astype(np.float32)
        wq6 = (w6 / sc[:, None, :, :, None, None]).astype(ml_dtypes.float8_e3m4)
        _HOST_SCALE[0] = sc  # [o, h, w]
        src, dt = wq6, ml_dtypes.float8_e3m4
    else:
        _HOST_SCALE[0] = None
        src, dt = w6, ml_dtypes.bfloat16

    # [h, ki, i, c, j3, o]; w = c-2+j3, kj = 2-j3, c = w+kj
    Wb = np.zeros((NCORES * RH, 3, CIN, W, 3, COUT), dt)
    for kj in range(3):
        slab = np.transpose(src[:, :, :, :, :, kj], (2, 4, 1, 3, 0))  # h,ki,i,w,o
        Wb[:OH, :, :, kj : OH + kj, 2 - kj, :] = slab.astype(dt)
    # -> per core [NQ=2, 96, pq=2, hp=2, W, 3, COUT]
    Wb = Wb.reshape(NCORES, NPAIR // 2, 2, 2, 96, W, 3, COUT)
    Wb = np.ascontiguousarray(np.transpose(Wb, (0, 1, 4, 2, 3, 5, 6, 7)))

    # ---- x: stacked rows, pair-major [NPAIR, 96, 2, B, W] ----
    x_fp8 = X_FP8 and not i8
    x_np_dt = ml_dtypes.float8_e3m4 if x_fp8 else ml_dtypes.bfloat16
    xpad = np.zeros((B, CIN, H + 2, W), np.float32)
    xpad[:, :, :H, :] = (x * X_SCALE) if x_fp8 else x

    in_maps = []
    for core in range(NCORES):
        r0 = RH * core
        xw = xpad[:, :, r0 : r0 + RH + 2, :]  # [b,i,RH+2,w]
        sv = np.lib.stride_tricks.sliding_window_view(xw, 3, axis=2)  # b,i,RH,w,ki
        # rows h-major -> [q, pq, hp, 96, B, W] -> [q, 96, pq, hp, B, W]
        xs_c = np.transpose(sv, (2, 4, 1, 0, 3)).reshape(NPAIR // 2, 2, 2, 96, B, W)
        xs_c = np.ascontiguousarray(
            np.transpose(xs_c, (0, 3, 1, 2, 4, 5)), dtype=x_np_dt
        )
        in_maps.append({"wt": Wb[core], "xs": xs_c})
    return in_maps


def kernel(x, weight):
    global LAST
    from concourse.bass_utils import run_bass_kernel_spmd

    if MODE not in _PROGRAMS:
        _PROGRAMS[MODE] = _build_program(mode=MODE)
    in_maps = _prep_inputs(np.asarray(x), np.asarray(weight))
    res = run_bass_kernel_spmd(
        _PROGRAMS[MODE], in_maps, list(range(NCORES)), trace=TRACE
    )
    LAST = res
    # per core out [NQ, 128, 2, OW, COUT] fp16 -> [b, o, h, w] f32
    full = np.stack([r["out"] for r in res.results])  # [8, 2, 128, 2, 62, 64]
    full = np.transpose(full, (0, 1, 3, 2, 4, 5))  # [8, q, pq, 128, 62, 64]
    arr = full.reshape(NCORES * NPAIR, 2, B, OW, COUT).astype(np.float32)
    # [(core,pair), hp, b, w, o] -> [b, o, (core,pair,hp), w]
    arr = np.transpose(arr, (2, 4, 0, 1, 3)).reshape(B, COUT, NCORES * RH, OW)
    arr = np.ascontiguousarray(arr[:, :, :OH])
    sc = _HOST_SCALE[0]
    if sc is not None:
        if X_FP8 and MODE == "fp8":
            sc = sc / np.float32(X_SCALE)
        arr *= sc[None]
    return arr



# revision 49
# speedup vs baseline: 2.2424x; 2.2424x over previous
"""LocallyConnected2d (3x3, stride 1) Trainium2 Bass kernel, v6 (fp8).

Shapes: x [64,32,64,64] f32, weight [1,64,32,62,62,9] f32 -> out [64,64,62,62] f32.

Strategy (orientation-B PE structure, fp8 everywhere the DMA bus is hot):
  - Shard output rows (OH=62, padded 64) across 8 cores: 8 rows/core,
    processed as 4 pairs (h even -> PE column group 0 / PSUM partitions
    0-63, h odd -> col group 1 / partitions 64-127).
  - Per x column c: stationary = X_c [96=(ki,i), 64=b] (LDWEIGHTS once),
    streamed = the weight block consuming x[:,c]: up to 192 columns
    (w=c-2..c × 64 couts). PSUM chunk (8 w = one 2KB bank) accumulates via
    pending-zero semantics (start=True only on the chunk's first matmul).
  - BOTH operands float8_e3m4 (4 mantissa bits), fed to the PE directly:
    weights with per-(o,h,w)-row scales (dequant on host: out *= scale),
    x scaled by a global power of two (2.0). rel l2 ~1.73e-2 (gate 2e-2).
    This keeps the DMA bus traffic at 1 byte/element with ZERO on-chip
    conversion cost (the v3 int8->bf16 SWDGE convert path paid the bf16
    size on the DMA bus and was ~2.1x slower end to end).
  - The kernel is DMA-bound (inbound ~12.3 MB/core/iter at an effective
    ~200-250 GB/s shared bus; PE busy ~41 us). Weight DMA is split across
    all three DGE queues (SP/Act/Pool), x on Act, out on Pool; the weight
    layout drops the 6/192 zero-padded (c,kj) combos (NCJ=186).
  - out fp16 [pair,(hp,b),w,o]; measured ~58-63 us/iter steady state vs
    137.7 us for the staged baseline.
"""

import sys

if "/opt/trn_rl_repo" not in sys.path:
    sys.path.insert(0, "/opt/trn_rl_repo")

import numpy as np

B = 64
CIN = 32
H = W = 64
OH = OW = 62
COUT = 64
NCORES = 8
RH = 8
NPAIR = 4

MODE = "fp8"
TRACE = False
LAST = None
FP8_MAX = 15.5  # ml_dtypes.finfo(float8_e3m4).max
X_FP8 = True  # ship x as float8_e3m4 (scaled by X_SCALE) instead of bf16
X_SCALE = 2.0  # power of two; host dequant folds 1/X_SCALE into output

_PROGRAMS = {}


def _build_program(repeat=1, mode=None, variant="full"):
    mode = mode or MODE
    import concourse.bacc as bacc
    import concourse.mybir as mybir
    from concourse.tile import TileContext

    fp32 = mybir.dt.float32
    fp16 = mybir.dt.float16
    bf16 = mybir.dt.bfloat16
    nc = bacc.Bacc(
        "TRN2", target_bir_lowering=False, debug=False, num_devices=NCORES
    )

    i8 = mode == "i8"
    fp8 = mode == "fp8"
    if i8:
        wdram_dt, wtile_dt = mybir.dt.int8, bf16
    elif fp8:
        wdram_dt = wtile_dt = mybir.dt.float8e3
    else:
        wdram_dt = wtile_dt = bf16
    # [pair][p=(ki,i)][hp][cj][o]: cj = flattened valid (c, j3) pairs
    # (w = c-2+j3 in [0, OW)); c-major, j3-minor. len = 186.
    CJ = [(c, j3) for c in range(W) for j3 in range(3) if 0 <= c - 2 + j3 < OW]
    NCJ = len(CJ)
    cj_base = {}
    cj_lo = {}
    for idx, (c, j3) in enumerate(CJ):
        if c not in cj_base:
            cj_base[c] = idx
            cj_lo[c] = j3
    wt = nc.declare_dram_parameter(
        "wt", [NPAIR, 96, 2, NCJ, COUT], wdram_dt, isOutput=False
    )
    # [pair][p=(ki,i)][hp][b][w]
    x_dt = mybir.dt.float8e3 if (X_FP8 and not i8) else bf16
    xs = nc.declare_dram_parameter("xs", [NPAIR, 96, 2, B, W], x_dt, isOutput=False)
    # [pair][(hp,b)][w][o]
    out = nc.declare_dram_parameter("out", [NPAIR, 128, OW, COUT], fp16, isOutput=True)

    # (w0, nw) chunks: one PSUM bank (8 w x 64 o fp32 = 2KB) each
    CHUNKS = [(w0, min(8, OW - w0)) for w0 in range(0, OW, 8)]

    with TileContext(nc) as tc:
        with (
            tc.tile_pool(name="wp", bufs=2) as wpool,
            tc.tile_pool(name="xp", bufs=2) as xpool,
            tc.tile_pool(name="op", bufs=2) as opool,
            tc.tile_pool(
                name="pp", bufs=8 if variant == "psum8" else 4, space="PSUM"
            ) as ppool,
        ):
            for pair in [pp_ for _ in range(repeat) for pp_ in range(NPAIR)]:
                if variant == "splitw":
                    xq = oq = nc.gpsimd
                else:
                    xq = nc.sync if variant in ("onequeue", "xsp") else nc.scalar
                    oq = nc.sync if variant == "onequeue" else nc.gpsimd
                xt = xpool.tile([96, 2, B, W], x_dt, tag="x")
                xq.dma_start(out=xt[:], in_=xs[pair])
                wtile = wpool.tile([96, 2, NCJ, COUT], wtile_dt, tag="w")
                # weight DMA split across all three DGE queues (SP/Act/Pool)
                s1, s2 = NCJ // 3, 2 * NCJ // 3
                nc.sync.dma_start(out=wtile[:, :, :s1], in_=wt[pair][:, :, :s1])
                nc.scalar.dma_start(
                    out=wtile[:, :, s1:s2], in_=wt[pair][:, :, s1:s2]
                )
                nc.gpsimd.dma_start(out=wtile[:, :, s2:], in_=wt[pair][:, :, s2:])
                ot = opool.tile([128, OW, COUT], fp16, tag="o")
                skip_out = variant == "noout"
                for w0, nw in CHUNKS:
                    ps = ppool.tile([128, 8, COUT], fp32, tag="ps")
                    clist = list(range(w0, min(w0 + nw + 2, W)))
                    if variant == "minpe":
                        clist = clist[:1]
                    for ci, c in enumerate(clist):
                        ws = max(w0, c - 2)
                        we = min(c, w0 + nw - 1)
                        if ws > we:
                            continue
                        j3a = ws - c + 2
                        j3b = we - c + 2
                        ca = cj_base[c] + (j3a - cj_lo[c])
                        cb = cj_base[c] + (j3b - cj_lo[c])
                        first = ci == 0
                        last = c == clist[-1] or variant == "minpe"
                        for hp in range(2):
                            pb = 64 * hp
                            nc.tensor.matmul(
                                ps[pb : pb + 64, ws - w0 : we - w0 + 1, :],
                                lhsT=xt[:, hp, :, c],
                                rhs=wtile[:, hp, ca : cb + 1, :],
                                start=first,
                                stop=last,
                                tile_position=(0, pb),
                            )
                    nc.vector.tensor_copy(
                        ot[:, w0 : w0 + nw, :], ps[:, 0:nw, :]
                    )
                if not skip_out:
                    oq.dma_start(out=out[pair], in_=ot[:])
    nc.compile()
    return nc


_HOST_SCALE = [None]  # set by _prep_inputs in i8 mode; [o, h, w] f32


def _prep_inputs(x, weight, mode=None):
    mode = mode or MODE
    import ml_dtypes

    x = np.ascontiguousarray(x, dtype=np.float32)
    weight = np.ascontiguousarray(weight, dtype=np.float32)
    i8 = mode == "i8"

    # ---- weights ----
    w6 = weight[0].reshape(COUT, CIN, OH, OW, 3, 3)  # o,i,h,w,ki,kj
    if i8:
        am = np.abs(w6).max(axis=(1, 4, 5))  # [o, h, w]
        am = np.maximum(am, 1e-30)
        q = 127.0 / am
        wq6 = (
            np.rint(w6 * q[:, None, :, :, None, None]).clip(-127, 127).astype(np.int8)
        )
        _HOST_SCALE[0] = (am / 127.0).astype(np.float32)  # [o, h, w]
        src, dt = wq6, np.int8
    elif mode == "fp8":
        am = np.abs(w6).max(axis=(1, 4, 5))  # [o, h, w]
        am = np.maximum(am, 1e-30)
        sc = (am / FP8_MAX).astype(np.float32)
        wq6 = (w6 / sc[:, None, :, :, None, None]).astype(ml_dtypes.float8_e3m4)
        _HOST_SCALE[0] = sc  # [o, h, w]
        src, dt = wq6, ml_dtypes.float8_e3m4
    else:
        _HOST_SCALE[0] = None
        src, dt = w6, ml_dtypes.bfloat16

    # [h, ki, i, c, j3, o]; w = c-2+j3, kj = 2-j3, c = w+kj
    Wb = np.zeros((NCORES * RH, 3, CIN, W, 3, COUT), dt)
    for kj in range(3):
        slab = np.transpose(src[:, :, :, :, :, kj], (2, 4, 1, 3, 0))  # h,ki,i,w,o
        Wb[:OH, :, :, kj : OH + kj, 2 - kj, :] = slab.astype(dt)
    # -> per core [NPAIR, 96, 2(hp), NCJ=186, COUT] (drop invalid (c,j3) pads)
    vidx = [3 * c + j3 for c in range(W) for j3 in range(3) if 0 <= c - 2 + j3 < OW]
    Wb = Wb.reshape(NCORES, NPAIR, 2, 96, W * 3, COUT)[:, :, :, :, vidx, :]
    Wb = np.ascontiguousarray(np.transpose(Wb, (0, 1, 3, 2, 4, 5)))

    # ---- x: stacked rows, pair-major [NPAIR, 96, 2, B, W] ----
    x_fp8 = X_FP8 and not i8
    x_np_dt = ml_dtypes.float8_e3m4 if x_fp8 else ml_dtypes.bfloat16
    xpad = np.zeros((B, CIN, H + 2, W), np.float32)
    xpad[:, :, :H, :] = (x * X_SCALE) if x_fp8 else x

    in_maps = []
    for core in range(NCORES):
        r0 = RH * core
        xw = xpad[:, :, r0 : r0 + RH + 2, :]  # [b,i,RH+2,w]
        sv = np.lib.stride_tricks.sliding_window_view(xw, 3, axis=2)  # b,i,RH,w,ki
        xs_c = np.transpose(sv, (2, 4, 1, 0, 3)).reshape(NPAIR, 2, 96, B, W)
        xs_c = np.ascontiguousarray(
            np.transpose(xs_c, (0, 2, 1, 3, 4)), dtype=x_np_dt
        )
        in_maps.append({"wt": Wb[core], "xs": xs_c})
    return in_maps


def kernel(x, weight):
    global LAST
    from concourse.bass_utils import run_bass_kernel_spmd

    if MODE not in _PROGRAMS:
        _PROGRAMS[MODE] = _build_program(mode=MODE)
    in_maps = _prep_inputs(np.asarray(x), np.asarray(weight))
    res = run_bass_kernel_spmd(
        _PROGRAMS[MODE], in_maps, list(range(NCORES)), trace=TRACE
    )
    LAST = res
    # per core out [NPAIR, 128, OW, COUT] fp16 -> [b, o, h, w] f32
    full = np.stack([r["out"] for r in res.results])  # [8, 4, 128, 62, 64]
    arr = full.reshape(NCORES * NPAIR, 2, B, OW, COUT).astype(np.float32)
    # [(core,pair), hp, b, w, o] -> [b, o, (core,pair,hp), w]
    arr = np.transpose(arr, (2, 4, 0, 1, 3)).reshape(B, COUT, NCORES * RH, OW)
    arr = np.ascontiguousarray(arr[:, :, :OH])
    sc = _HOST_SCALE[0]
    if sc is not None:
        if X_FP8 and MODE == "fp8":
            sc = sc / np.float32(X_SCALE)
        arr *= sc[None]
    return arr



# revision 53
# speedup vs baseline: 2.2863x; 1.0196x over previous
"""LocallyConnected2d (3x3, stride 1) Trainium2 Bass kernel, v6 (fp8).

Shapes: x [64,32,64,64] f32, weight [1,64,32,62,62,9] f32 -> out [64,64,62,62] f32.

Strategy (orientation-B PE structure, fp8 everywhere the DMA bus is hot):
  - Shard output rows (OH=62, padded 64) across 8 cores: 8 rows/core,
    processed as 4 pairs (h even -> PE column group 0 / PSUM partitions
    0-63, h odd -> col group 1 / partitions 64-127).
  - Per x column c: stationary = X_c [96=(ki,i), 64=b] (LDWEIGHTS once),
    streamed = the weight block consuming x[:,c]: up to 192 columns
    (w=c-2..c × 64 couts). PSUM chunk (8 w = one 2KB bank) accumulates via
    pending-zero semantics (start=True only on the chunk's first matmul).
  - BOTH operands float8_e3m4 (4 mantissa bits), fed to the PE directly:
    weights with per-(o,h,w)-row scales (dequant on host: out *= scale),
    x scaled by a global power of two (2.0). rel l2 ~1.73e-2 (gate 2e-2).
    This keeps the DMA bus traffic at 1 byte/element with ZERO on-chip
    conversion cost (the v3 int8->bf16 SWDGE convert path paid the bf16
    size on the DMA bus and was ~2.1x slower end to end).
  - The kernel is DMA-bound (inbound ~12.3 MB/core/iter at an effective
    ~200-250 GB/s shared bus; PE busy ~41 us). Weight DMA is split across
    all three DGE queues (SP/Act/Pool), x on Act, out on Pool; the weight
    layout drops the 6/192 zero-padded (c,kj) combos (NCJ=186).
  - out fp16 [pair,(hp,b),w,o]; measured ~58-63 us/iter steady state vs
    137.7 us for the staged baseline.
"""

import sys

if "/opt/trn_rl_repo" not in sys.path:
    sys.path.insert(0, "/opt/trn_rl_repo")

import numpy as np

B = 64
CIN = 32
H = W = 64
OH = OW = 62
COUT = 64
NCORES = 8
RH = 8
NPAIR = 4

MODE = "fp8"
TRACE = False
LAST = None
FP8_MAX = 15.5  # ml_dtypes.finfo(float8_e3m4).max
X_FP8 = True  # ship x as float8_e3m4 (scaled by X_SCALE) instead of bf16
X_SCALE = 2.0  # power of two; host dequant folds 1/X_SCALE into output

_PROGRAMS = {}


def _build_program(repeat=1, mode=None, variant="full"):
    mode = mode or MODE
    import concourse.bacc as bacc
    import concourse.mybir as mybir
    from concourse.tile import TileContext

    fp32 = mybir.dt.float32
    fp16 = mybir.dt.float16
    bf16 = mybir.dt.bfloat16
    nc = bacc.Bacc(
        "TRN2", target_bir_lowering=False, debug=False, num_devices=NCORES
    )

    i8 = mode == "i8"
    fp8 = mode == "fp8"
    if i8:
        wdram_dt, wtile_dt = mybir.dt.int8, bf16
    elif fp8:
        wdram_dt = wtile_dt = mybir.dt.float8e3
    else:
        wdram_dt = wtile_dt = bf16
    # [pair][p=(ki,i)][hp][cj][o]: cj = flattened valid (c, j3) pairs
    # (w = c-2+j3 in [0, OW)); c-major, j3-minor. len = 186.
    CJ = [(c, j3) for c in range(W) for j3 in range(3) if 0 <= c - 2 + j3 < OW]
    NCJ = len(CJ)
    cj_base = {}
    cj_lo = {}
    for idx, (c, j3) in enumerate(CJ):
        if c not in cj_base:
            cj_base[c] = idx
            cj_lo[c] = j3
    wt = nc.declare_dram_parameter(
        "wt", [NPAIR, 96, 2, NCJ, COUT], wdram_dt, isOutput=False
    )
    # [pair][p=(ki,i)][hp][b][w]
    x_dt = mybir.dt.float8e3 if (X_FP8 and not i8) else bf16
    xs = nc.declare_dram_parameter("xs", [NPAIR, 96, 2, B, W], x_dt, isOutput=False)
    # [pair][(hp,b)][w][o]
    out = nc.declare_dram_parameter("out", [NPAIR, 128, OW, COUT], fp16, isOutput=True)

    # (w0, nw) chunks: one PSUM bank (8 w x 64 o fp32 = 2KB) each
    CHUNKS = [(w0, min(8, OW - w0)) for w0 in range(0, OW, 8)]

    with TileContext(nc) as tc:
        with (
            tc.tile_pool(name="wp", bufs=2) as wpool,
            tc.tile_pool(name="xp", bufs=2) as xpool,
            tc.tile_pool(name="op", bufs=2) as opool,
            tc.tile_pool(
                name="pp", bufs=8 if variant == "psum8" else 4, space="PSUM"
            ) as ppool,
        ):
            for pair in [pp_ for _ in range(repeat) for pp_ in range(NPAIR)]:
                if variant == "splitw":
                    xq = oq = nc.gpsimd
                else:
                    xq = nc.sync if variant in ("onequeue", "xsp") else nc.scalar
                    oq = (
                        nc.sync
                        if variant in ("onequeue", "outsp")
                        else nc.gpsimd
                    )
                xt = xpool.tile([96, 2, B, W], x_dt, tag="x")
                xq.dma_start(out=xt[:], in_=xs[pair])
                wtile = wpool.tile([96, 2, NCJ, COUT], wtile_dt, tag="w")
                # weight DMA split across all three DGE queues (SP/Act/Pool)
                # SP 1/4, Act 1/4, Pool 1/2 measured best (Pool/SWDGE queue
                # has partially independent inbound DMA capacity)
                s1, s2 = NCJ // 4, NCJ // 2
                nc.sync.dma_start(out=wtile[:, :, :s1], in_=wt[pair][:, :, :s1])
                nc.scalar.dma_start(
                    out=wtile[:, :, s1:s2], in_=wt[pair][:, :, s1:s2]
                )
                nc.gpsimd.dma_start(out=wtile[:, :, s2:], in_=wt[pair][:, :, s2:])
                ot = opool.tile([128, OW, COUT], fp16, tag="o")
                skip_out = variant == "noout"
                for w0, nw in CHUNKS:
                    ps = ppool.tile([128, 8, COUT], fp32, tag="ps")
                    clist = list(range(w0, min(w0 + nw + 2, W)))
                    if variant == "minpe":
                        clist = clist[:1]
                    for ci, c in enumerate(clist):
                        ws = max(w0, c - 2)
                        we = min(c, w0 + nw - 1)
                        if ws > we:
                            continue
                        j3a = ws - c + 2
                        j3b = we - c + 2
                        ca = cj_base[c] + (j3a - cj_lo[c])
                        cb = cj_base[c] + (j3b - cj_lo[c])
                        first = ci == 0
                        last = c == clist[-1] or variant == "minpe"
                        for hp in range(2):
                            pb = 64 * hp
                            nc.tensor.matmul(
                                ps[pb : pb + 64, ws - w0 : we - w0 + 1, :],
                                lhsT=xt[:, hp, :, c],
                                rhs=wtile[:, hp, ca : cb + 1, :],
                                start=first,
                                stop=last,
                                tile_position=(0, pb),
                            )
                    nc.vector.tensor_copy(
                        ot[:, w0 : w0 + nw, :], ps[:, 0:nw, :]
                    )
                if not skip_out:
                    oq.dma_start(out=out[pair], in_=ot[:])
    nc.compile()
    return nc


_HOST_SCALE = [None]  # set by _prep_inputs in i8 mode; [o, h, w] f32


def _prep_inputs(x, weight, mode=None):
    mode = mode or MODE
    import ml_dtypes

    x = np.ascontiguousarray(x, dtype=np.float32)
    weight = np.ascontiguousarray(weight, dtype=np.float32)
    i8 = mode == "i8"

    # ---- weights ----
    w6 = weight[0].reshape(COUT, CIN, OH, OW, 3, 3)  # o,i,h,w,ki,kj
    if i8:
        am = np.abs(w6).max(axis=(1, 4, 5))  # [o, h, w]
        am = np.maximum(am, 1e-30)
        q = 127.0 / am
        wq6 = (
            np.rint(w6 * q[:, None, :, :, None, None]).clip(-127, 127).astype(np.int8)
        )
        _HOST_SCALE[0] = (am / 127.0).astype(np.float32)  # [o, h, w]
        src, dt = wq6, np.int8
    elif mode == "fp8":
        am = np.abs(w6).max(axis=(1, 4, 5))  # [o, h, w]
        am = np.maximum(am, 1e-30)
        sc = (am / FP8_MAX).astype(np.float32)
        wq6 = (w6 / sc[:, None, :, :, None, None]).astype(ml_dtypes.float8_e3m4)
        _HOST_SCALE[0] = sc  # [o, h, w]
        src, dt = wq6, ml_dtypes.float8_e3m4
    else:
        _HOST_SCALE[0] = None
        src, dt = w6, ml_dtypes.bfloat16

    # [h, ki, i, c, j3, o]; w = c-2+j3, kj = 2-j3, c = w+kj
    Wb = np.zeros((NCORES * RH, 3, CIN, W, 3, COUT), dt)
    for kj in range(3):
        slab = np.transpose(src[:, :, :, :, :, kj], (2, 4, 1, 3, 0))  # h,ki,i,w,o
        Wb[:OH, :, :, kj : OH + kj, 2 - kj, :] = slab.astype(dt)
    # -> per core [NPAIR, 96, 2(hp), NCJ=186, COUT] (drop invalid (c,j3) pads)
    vidx = [3 * c + j3 for c in range(W) for j3 in range(3) if 0 <= c - 2 + j3 < OW]
    Wb = Wb.reshape(NCORES, NPAIR, 2, 96, W * 3, COUT)[:, :, :, :, vidx, :]
    Wb = np.ascontiguousarray(np.transpose(Wb, (0, 1, 3, 2, 4, 5)))

    # ---- x: stacked rows, pair-major [NPAIR, 96, 2, B, W] ----
    x_fp8 = X_FP8 and not i8
    x_np_dt = ml_dtypes.float8_e3m4 if x_fp8 else ml_dtypes.bfloat16
    xpad = np.zeros((B, CIN, H + 2, W), np.float32)
    xpad[:, :, :H, :] = (x * X_SCALE) if x_fp8 else x

    in_maps = []
    for core in range(NCORES):
        r0 = RH * core
        xw = xpad[:, :, r0 : r0 + RH + 2, :]  # [b,i,RH+2,w]
        sv = np.lib.stride_tricks.sliding_window_view(xw, 3, axis=2)  # b,i,RH,w,ki
        xs_c = np.transpose(sv, (2, 4, 1, 0, 3)).reshape(NPAIR, 2, 96, B, W)
        xs_c = np.ascontiguousarray(
            np.transpose(xs_c, (0, 2, 1, 3, 4)), dtype=x_np_dt
        )
        in_maps.append({"wt": Wb[core], "xs": xs_c})
    return in_maps


def kernel(x, weight):
    global LAST
    from concourse.bass_utils import run_bass_kernel_spmd

    if MODE not in _PROGRAMS:
        _PROGRAMS[MODE] = _build_program(mode=MODE)
    in_maps = _prep_inputs(np.asarray(x), np.asarray(weight))
    res = run_bass_kernel_spmd(
        _PROGRAMS[MODE], in_maps, list(range(NCORES)), trace=TRACE
    )
    LAST = res
    # per core out [NPAIR, 128, OW, COUT] fp16 -> [b, o, h, w] f32
    full = np.stack([r["out"] for r in res.results])  # [8, 4, 128, 62, 64]
    arr = full.reshape(NCORES * NPAIR, 2, B, OW, COUT).astype(np.float32)
    # [(core,pair), hp, b, w, o] -> [b, o, (core,pair,hp), w]
    arr = np.transpose(arr, (2, 4, 0, 1, 3)).reshape(B, COUT, NCORES * RH, OW)
    arr = np.ascontiguousarray(arr[:, :, :OH])
    sc = _HOST_SCALE[0]
    if sc is not None:
        if X_FP8 and MODE == "fp8":
            sc = sc / np.float32(X_SCALE)
        arr *= sc[None]
    return arr

